# revision 33
# baseline (speedup 1.0000x reference)
"""Trainium2 Bass kernel for nn_Blur: 4x4 FIR depthwise blur with pad (2,1).

out[n,c,i,j] = sum_{a,b} K[a,b] * x[n,c, i+1-a, j+1-b]   (zero-padded)

Strategy (8 NeuronCores, pure data parallelism over the 8192 (n,c) slices):
  - Each core processes 1024 slices of 64x64.
  - SBUF layout per tile of 64 slices: partition p = m*64 + h (member
    m = slice parity packs two slices so the full 128-wide PE contraction
    is used), free = (s, w) with w zero-padded to 67 (2 left + 1 right).
  - The H-convolution lives in 4 banded [128,128] stationary matrices (one
    per W-tap b): lhsT_b[u + 64m, i + 64m'] = delta(m,m') * K[i-u+1, b].
  - The W-convolution comes from 4 PSUM-accumulated matmuls whose rhs is
    the same tile shifted along the free (W) dim; the zero pad makes all
    four matmuls full-range N=512.
  - float32r matmuls run at full PE rate for N>=256; inputs are declared
    float32r (same bits host-side) so every DMA is same-dtype.
  - The host pre-permutes each core's shard into the exact SBUF tile
    layout, so every DMA descriptor is one contiguous run per partition
    (line-rate HBM instead of 256B-descriptor-limited).
"""

import numpy as np

import concourse.bacc as bacc
import concourse.mybir as mybir
from concourse.tile import TileContext
from concourse.bass_utils import run_bass_kernel_spmd

N_CORES = 8
B, C, H, W = 32, 256, 64, 64
NSLICES = B * C                      # 8192
SLICES_PER_CORE = NSLICES // N_CORES  # 1024
TILE_SLICES = 64                     # slices per SBUF tile (2 members x 32)
SG = TILE_SLICES // 2                # s-groups per member = 32
WPAD = W + 3                         # 2 left zero cols + 1 right zero col
F32 = mybir.dt.float32
F32R = mybir.dt.float32r

_NC_CACHE = {}


def _build_wmat(K: np.ndarray) -> np.ndarray:
    """(4, 128, 128) fp32: per-W-tap block-diag transposed H-band matrices."""
    K = np.asarray(K, np.float32)
    wmat = np.zeros((4, 128, 128), np.float32)
    for b in range(4):
        T = np.zeros((H, H), np.float32)
        for i in range(H):
            for u in range(max(0, i - 2), min(H, i + 2)):
                T[i, u] = K[i - u + 1, b]
        lhsT = T.T  # lhsT[u, i] = K[i-u+1, b]
        wmat[b, :H, :H] = lhsT
        wmat[b, H:, H:] = lhsT
    return wmat


WARMUP_MMS = 0


def _build_nc(slices_per_core: int = SLICES_PER_CORE):
    ntiles = slices_per_core // TILE_SLICES
    nc = bacc.Bacc("TRN2", target_bir_lowering=False, debug=False)
    # DRAM layouts are the SBUF tile layouts (host pre-/post-permutes):
    #   x: [tile, p=(m h), (s w')]  with w' zero-padded to WPAD — fp32r
    #   matmuls require full-range contiguous dst patterns, so the W-tap
    #   shifts come from the padded rhs, never from truncated out APs
    #   (walrus s3d3_mm_fp32r_restrictions).
    x = nc.dram_tensor(
        "x", [ntiles, 128, SG * WPAD], F32R, kind="ExternalInput"
    ).ap()
    wm = nc.dram_tensor("w", [4, 128, 128], F32R, kind="ExternalInput").ap()
    y = nc.dram_tensor(
        "y", [ntiles, 128, SG * W], F32, kind="ExternalOutput"
    ).ap()
    # sink for the PE warm-up matmuls (kept alive so DCE can't drop them)
    warm_out = nc.dram_tensor("warm", [128, 4], F32, kind="ExternalOutput").ap()

    with TileContext(nc) as tc:
        with (
            tc.tile_pool(name="wpool", bufs=1) as wpool,
            tc.tile_pool(name="xpool", bufs=6) as xpool,
            tc.tile_pool(name="opool", bufs=6) as opool,
            tc.tile_pool(name="pspool", bufs=8, space="PSUM") as pspool,
        ):
            wsb = wpool.tile([128, 4, 128], F32R, name="wsb")
            nc.sync.dma_start(wsb[:], wm.rearrange("b k m -> k b m"))

            # PE warm-up: ~4us of matmuls on the weight tile while the first
            # input tiles stream in, so the HAM clock gate opens (1.2 ->
            # 2.4 GHz) before the real matmuls start. Only depends on wsb.
            wscratch = wpool.tile([128, 4], F32, name="wscratch")
            if WARMUP_MMS:
                wps = pspool.tile([128, 512], F32, name="wps", tag="ps")
                for r in range(WARMUP_MMS):
                    nc.tensor.matmul(
                        wps[:],
                        wsb[:, 0, :],
                        wsb.rearrange("p b m -> p (b m)"),
                        start=(r == 0),
                        stop=(r == WARMUP_MMS - 1),
                    )
                nc.vector.tensor_copy(wscratch[:], wps[:, 0:4])
            else:
                nc.vector.tensor_copy(wscratch[:], wsb[:, 0, 0:4].bitcast(F32))
            nc.sync.dma_start(warm_out, wscratch[:])

            for t in range(ntiles):
                xt = xpool.tile([128, SG, WPAD], F32R, name="xt")
                nc.sync.dma_start(xt[:], x[t])

                ot = opool.tile([128, SG, W], F32, name="ot")
                for q in range(SG // 8):
                    ps = pspool.tile([128, 512], F32, name="ps")
                    for b in range(4):
                        c0 = 3 - b
                        nc.tensor.matmul(
                            ps[:],
                            wsb[:, b, :],
                            xt[:, 8 * q : 8 * q + 8, c0 : c0 + W],
                            start=(b == 0),
                            stop=(b == 3),
                        )
                    # alternate copy engine: DVE and ACT are both idle-ish
                    if q % 2 == 0:
                        nc.vector.tensor_copy(ot[:, 8 * q : 8 * q + 8, :], ps[:])
                    else:
                        nc.scalar.copy(ot[:, 8 * q : 8 * q + 8, :], ps[:])

                    # half-tile output stores on the ACT HWDGE ring: stores
                    # start earlier and never head-of-line-block the SP ring
                    if q == 1:
                        nc.scalar.dma_start(
                            y[t][:, : SG * W // 2], ot[:, : SG // 2, :]
                        )
                    elif q == 3:
                        nc.scalar.dma_start(
                            y[t][:, SG * W // 2 :], ot[:, SG // 2 :, :]
                        )

    nc.compile()
    return nc


def get_nc(slices_per_core: int = SLICES_PER_CORE):
    if slices_per_core not in _NC_CACHE:
        _NC_CACHE[slices_per_core] = _build_nc(slices_per_core)
    return _NC_CACHE[slices_per_core]


def _pack_input(xs: np.ndarray) -> np.ndarray:
    """[S, H, W] fp32 -> [S/64, 128, SG*WPAD] in the SBUF tile layout."""
    s = xs.shape[0]
    ntiles = s // TILE_SLICES
    xp = np.zeros((s, H, WPAD), np.float32)
    xp[:, :, 2 : 2 + W] = xs
    # (t, s, m, h, w) -> (t, m, h, s, w)
    v = xp.reshape(ntiles, SG, 2, H, WPAD).transpose(0, 2, 3, 1, 4)
    return np.ascontiguousarray(v.reshape(ntiles, 128, SG * WPAD))


def _unpack_output(yp: np.ndarray) -> np.ndarray:
    """[S/64, 128, SG*W] -> [S, H, W]."""
    ntiles = yp.shape[0]
    v = yp.reshape(ntiles, 2, H, SG, W).transpose(0, 3, 1, 2, 4)
    return v.reshape(ntiles * TILE_SLICES, H, W)


def kernel(x: np.ndarray, kernel: np.ndarray, _trace: bool = False, **_tkw):
    x = np.asarray(x, np.float32)
    wmat = _build_wmat(kernel)
    b, c, h, w = x.shape
    xs = x.reshape(b * c, h, w)
    spc = (b * c) // N_CORES
    nc = get_nc(spc)
    in_maps = [
        {"x": _pack_input(xs[k * spc : (k + 1) * spc]), "w": wmat}
        for k in range(N_CORES)
    ]
    res = run_bass_kernel_spmd(
        nc, in_maps, list(range(N_CORES)), trace=_trace, **_tkw
    )
    out = np.concatenate(
        [_unpack_output(res.results[k]["y"]) for k in range(N_CORES)], axis=0
    )
    result = out.reshape(b, c, h, w)
    if _trace:
        return result, res
    return result


# revision 35
# speedup vs baseline: 1.1104x; 1.1104x over previous
"""Trainium2 Bass kernel for nn_Blur: 4x4 FIR depthwise blur with pad (2,1).

out[n,c,i,j] = sum_{a,b} K[a,b] * x[n,c, i+1-a, j+1-b]   (zero-padded)

Strategy (8 NeuronCores, pure data parallelism over the 8192 (n,c) slices):
  - Each core processes 1024 slices of 64x64.
  - SBUF layout per tile of 64 slices: partition p = m*64 + h (member
    m = slice parity packs two slices so the full 128-wide PE contraction
    is used), free = (s, w) with w zero-padded to 67 (2 left + 1 right).
  - The H-convolution lives in 4 banded [128,128] stationary matrices (one
    per W-tap b): lhsT_b[u + 64m, i + 64m'] = delta(m,m') * K[i-u+1, b].
  - The W-convolution comes from 4 PSUM-accumulated matmuls whose rhs is
    the same tile shifted along the free (W) dim; the zero pad makes all
    four matmuls full-range N=512.
  - float32r matmuls run at full PE rate for N>=256; inputs are declared
    float32r (same bits host-side) so every DMA is same-dtype.
  - The host pre-permutes each core's shard into the exact SBUF tile
    layout, so every DMA descriptor is one contiguous run per partition
    (line-rate HBM instead of 256B-descriptor-limited).
"""

import sys
import types

import numpy as np

import concourse.bacc as bacc
import concourse.mybir as mybir
from concourse.tile import TileContext
from concourse.bass_utils import run_bass_kernel_spmd


def _install_ntff_hook():
    """Best-effort shim: this image's antenv lacks axon_hooks, which the
    trace=True path of run_bass_kernel_spmd imports. Harmless if unused."""
    if "antenv.axon_hooks" in sys.modules:
        return
    try:
        sys.path.insert(0, "/root/.axon_site")
        from trn_agent_boot.trn_boot import _ntff_profile_via_ctypes

        hook = _ntff_profile_via_ctypes("/opt/axon/libaxon_pjrt.so")
        mod = types.ModuleType("antenv.axon_hooks")
        mod.get_axon_ntff_profile_hook = lambda: hook
        mod.set_axon_ntff_profile_hook = lambda h: None
        sys.modules["antenv.axon_hooks"] = mod
    except Exception:
        pass


_install_ntff_hook()

N_CORES = 8
B, C, H, W = 32, 256, 64, 64
NSLICES = B * C                      # 8192
SLICES_PER_CORE = NSLICES // N_CORES  # 1024
TILE_SLICES = 64                     # slices per SBUF tile (2 members x 32)
SG = TILE_SLICES // 2                # s-groups per member = 32
WPAD = W + 3                         # 2 left zero cols + 1 right zero col
F32 = mybir.dt.float32
F32R = mybir.dt.float32r

_NC_CACHE = {}


def _build_wmat(K: np.ndarray) -> np.ndarray:
    """(4, 128, 128) fp32: per-W-tap block-diag transposed H-band matrices."""
    K = np.asarray(K, np.float32)
    wmat = np.zeros((4, 128, 128), np.float32)
    for b in range(4):
        T = np.zeros((H, H), np.float32)
        for i in range(H):
            for u in range(max(0, i - 2), min(H, i + 2)):
                T[i, u] = K[i - u + 1, b]
        lhsT = T.T  # lhsT[u, i] = K[i-u+1, b]
        wmat[b, :H, :H] = lhsT
        wmat[b, H:, H:] = lhsT
    return wmat


WARMUP_MMS = 10


def _build_nc(slices_per_core: int = SLICES_PER_CORE):
    ntiles = slices_per_core // TILE_SLICES
    nc = bacc.Bacc("TRN2", target_bir_lowering=False, debug=False)
    # DRAM layouts are the SBUF tile layouts (host pre-/post-permutes):
    #   x: [tile, p=(m h), (s w')]  with w' zero-padded to WPAD — fp32r
    #   matmuls require full-range contiguous dst patterns, so the W-tap
    #   shifts come from the padded rhs, never from truncated out APs
    #   (walrus s3d3_mm_fp32r_restrictions).
    x = nc.dram_tensor(
        "x", [ntiles, 128, SG * WPAD], F32R, kind="ExternalInput"
    ).ap()
    wm = nc.dram_tensor("w", [4, 128, 128], F32R, kind="ExternalInput").ap()
    y = nc.dram_tensor(
        "y", [ntiles, 128, SG * W], F32, kind="ExternalOutput"
    ).ap()
    # sink for the PE warm-up matmuls (kept alive so DCE can't drop them)
    warm_out = nc.dram_tensor("warm", [128, 4], F32, kind="ExternalOutput").ap()

    with TileContext(nc) as tc:
        with (
            tc.tile_pool(name="wpool", bufs=1) as wpool,
            tc.tile_pool(name="xpool", bufs=6) as xpool,
            tc.tile_pool(name="opool", bufs=6) as opool,
            tc.tile_pool(name="pspool", bufs=8, space="PSUM") as pspool,
        ):
            wsb = wpool.tile([128, 4, 128], F32R, name="wsb")
            nc.sync.dma_start(wsb[:], wm.rearrange("b k m -> k b m"))

            # PE warm-up: ~4us of matmuls on the weight tile while the first
            # input tiles stream in, so the HAM clock gate opens (1.2 ->
            # 2.4 GHz) before the real matmuls start. Only depends on wsb.
            wscratch = wpool.tile([128, 4], F32, name="wscratch")
            if WARMUP_MMS:
                wps = pspool.tile([128, 512], F32, name="wps", tag="ps")
                for r in range(WARMUP_MMS):
                    nc.tensor.matmul(
                        wps[:],
                        wsb[:, 0, :],
                        wsb.rearrange("p b m -> p (b m)"),
                        start=(r == 0),
                        stop=(r == WARMUP_MMS - 1),
                    )
                nc.vector.tensor_copy(wscratch[:], wps[:, 0:4])
            else:
                nc.vector.tensor_copy(wscratch[:], wsb[:, 0, 0:4].bitcast(F32))
            nc.sync.dma_start(warm_out, wscratch[:])

            for t in range(ntiles):
                xt = xpool.tile([128, SG, WPAD], F32R, name="xt")
                nc.sync.dma_start(xt[:], x[t])

                ot = opool.tile([128, SG, W], F32, name="ot")
                for q in range(SG // 8):
                    ps = pspool.tile([128, 512], F32, name="ps")
                    for b in range(4):
                        c0 = 3 - b
                        nc.tensor.matmul(
                            ps[:],
                            wsb[:, b, :],
                            xt[:, 8 * q : 8 * q + 8, c0 : c0 + W],
                            start=(b == 0),
                            stop=(b == 3),
                        )
                    # alternate copy engine: DVE and ACT are both idle-ish
                    if q % 2 == 0:
                        nc.vector.tensor_copy(ot[:, 8 * q : 8 * q + 8, :], ps[:])
                    else:
                        nc.scalar.copy(ot[:, 8 * q : 8 * q + 8, :], ps[:])

                    # half-tile output stores on the ACT HWDGE ring: stores
                    # start earlier and never head-of-line-block the SP ring
                    if q == 1:
                        nc.scalar.dma_start(
                            y[t][:, : SG * W // 2], ot[:, : SG // 2, :]
                        )
                    elif q == 3:
                        nc.scalar.dma_start(
                            y[t][:, SG * W // 2 :], ot[:, SG // 2 :, :]
                        )

    nc.compile()
    return nc


def get_nc(slices_per_core: int = SLICES_PER_CORE):
    if slices_per_core not in _NC_CACHE:
        _NC_CACHE[slices_per_core] = _build_nc(slices_per_core)
    return _NC_CACHE[slices_per_core]


def _pack_input(xs: np.ndarray) -> np.ndarray:
    """[S, H, W] fp32 -> [S/64, 128, SG*WPAD] in the SBUF tile layout."""
    s = xs.shape[0]
    ntiles = s // TILE_SLICES
    xp = np.zeros((s, H, WPAD), np.float32)
    xp[:, :, 2 : 2 + W] = xs
    # (t, s, m, h, w) -> (t, m, h, s, w)
    v = xp.reshape(ntiles, SG, 2, H, WPAD).transpose(0, 2, 3, 1, 4)
    return np.ascontiguousarray(v.reshape(ntiles, 128, SG * WPAD))


def _unpack_output(yp: np.ndarray) -> np.ndarray:
    """[S/64, 128, SG*W] -> [S, H, W]."""
    ntiles = yp.shape[0]
    v = yp.reshape(ntiles, 2, H, SG, W).transpose(0, 3, 1, 2, 4)
    return v.reshape(ntiles * TILE_SLICES, H, W)


def kernel(x: np.ndarray, kernel: np.ndarray, _trace: bool = False, **_tkw):
    x = np.asarray(x, np.float32)
    wmat = _build_wmat(kernel)
    b, c, h, w = x.shape
    xs = x.reshape(b * c, h, w)
    spc = (b * c) // N_CORES
    nc = get_nc(spc)
    in_maps = [
        {"x": _pack_input(xs[k * spc : (k + 1) * spc]), "w": wmat}
        for k in range(N_CORES)
    ]
    res = run_bass_kernel_spmd(
        nc, in_maps, list(range(N_CORES)), trace=_trace, **_tkw
    )
    out = np.concatenate(
        [_unpack_output(res.results[k]["y"]) for k in range(N_CORES)], axis=0
    )
    result = out.reshape(b, c, h, w)
    if _trace:
        return result, res
    return result
